# revision 21
# baseline (speedup 1.0000x reference)
"""AdditiveAttention Bass kernel for 8 Trainium2 NeuronCores.

Math (reference):
    q = queries @ W_q            [B,Q,H]
    k = keys @ W_k               [B,K,H]
    scores[b,q,k] = sum_h w_v[h] * tanh(q[b,q,h] + k[b,k,h])
    attn = softmax(mask(scores)) over K
    out = attn @ values          [B,Q,D]

Key structural choices (see repo notes):
  * Masked keys (k >= valid_len[b]) contribute exactly 0 to the softmax, so
    work is skipped at task granularity; valid_lens is host-visible inside
    kernel(), and the work list is built at host (compile) time.
  * |scores| <= ||w_v||_1 ~= 13 so softmax needs no max-subtraction; the
    per-chunk partials (o = sum exp(s)*v, z = sum exp(s)) are linear and are
    summed on host.
  * Work is packed into uniform (batch, key-chunk-of-C) tasks spread over the
    8 cores -> single SPMD program, near-perfect load balance.

Per-task device pipeline (C = 256 keys/task, split into 2 chunks of 128 for
partition-dim stages):
    PE : q_proj/k_proj projections (H on partitions)   [pipelined 1 task ahead]
    DVE: qk[h, q, c] = k_proj[h, c] + q_proj[h, q]     (per-partition scalar)
    ACT: feat = tanh(qk) -> bf16, flat 8K-element instructions
    PE : scoresT[c, q] = feat.T @ w_v                  (feat as stationary)
    ACT: p = exp(scoresT)
    PE : o[d, q] = V.T @ p ; z[q] = mask.T @ p         (mask via zeroed V rows)
Host: out[b] = (sum_t o_t) / (sum_t z_t).
"""

import math
from contextlib import ExitStack

import numpy as np
import ml_dtypes

import concourse.bass as bass
import concourse.mybir as mybir
import concourse.tile as tile
from concourse import bacc, bass_utils

F32 = mybir.dt.float32
BF16 = mybir.dt.bfloat16

B, Q, K, D, H = 16, 64, 1024, 256, 256
C = 256          # keys per task
CH = C // 128    # c chunks per task (partition-dim stages)
GQ = 16          # queries per tanh group
N_CORES = 8
DC = D // 128    # d chunks (2)
HC = H // 128    # h chunks (2)
# per-hh psum region width for projections (q | k | pad): keep each hh region
# inside one 2KB PSUM bank
PROJ_W = (Q + C) if HC * (Q + C) * 4 <= 2048 else 512


def emit_kernel(tc, aps, n_tasks):
    """Emit the per-core SPMD program for n_tasks uniform tasks."""
    nc = tc.nc
    ctx = tc.ctx

    keysT = aps["keysT"]        # [T, 128, DC, C] f32   (dp, dc, c)
    queriesT = aps["queriesT"]  # [T, 128, DC, Q] f32
    vals = aps["vals"]          # [T, 128, CH, D] f32   (cp, ch, d)
    maskv = aps["maskv"]        # [T, 128, CH] f32
    Wq = aps["Wq"]              # [128, DC, H] f32      (dp, dc, h)
    Wk = aps["Wk"]              # [128, DC, H] f32
    wv = aps["wv"]              # [128, HC] bf16
    o_out = aps["o_out"]        # [T, 128, DC, Q] f32   (dp, dc, q)
    s_out = aps["s_out"]        # [T, 1, Q] f32

    const_pool = ctx.enter_context(tc.tile_pool(name="const", bufs=1))
    in_pool = ctx.enter_context(tc.tile_pool(name="inp", bufs=2))
    proj_pool = ctx.enter_context(tc.tile_pool(name="proj", bufs=2))
    qk_pool = ctx.enter_context(tc.tile_pool(name="qk", bufs=3))
    feat_pool = ctx.enter_context(tc.tile_pool(name="feat", bufs=4))
    p_pool = ctx.enter_context(tc.tile_pool(name="p", bufs=2))
    out_pool = ctx.enter_context(tc.tile_pool(name="outp", bufs=2))
    ps_proj = ctx.enter_context(tc.tile_pool(name="psproj", bufs=2, space="PSUM"))
    ps_sc = ctx.enter_context(tc.tile_pool(name="pssc", bufs=2, space="PSUM"))
    ps_o = ctx.enter_context(tc.tile_pool(name="pso", bufs=2, space="PSUM"))

    # Resident constants.
    Wq_sb = const_pool.tile([128, DC, H], F32, tag="wq")
    Wk_sb = const_pool.tile([128, DC, H], F32, tag="wk")
    wv_sb = const_pool.tile([128, HC], BF16, tag="wv")
    nc.gpsimd.dma_start(Wq_sb[:], Wq[:])
    nc.gpsimd.dma_start(Wk_sb[:], Wk[:])
    nc.gpsimd.dma_start(wv_sb[:], wv[:])

    def emit_inputs_and_proj(t):
        """DMA inputs + projections + evacuation for task t."""
        k_sb = in_pool.tile([128, DC, C], F32, tag="k")
        qT_sb = in_pool.tile([128, DC, Q], F32, tag="q")
        v_sb = in_pool.tile([128, CH, D], F32, tag="v")
        m_sb = in_pool.tile([128, CH], F32, tag="m")
        # Small/early-needed tensors first; keys split per d-chunk so the
        # first k-proj matmul starts after half the transfer.
        nc.sync.dma_start(qT_sb[:], queriesT[t])
        nc.sync.dma_start(k_sb[:, 0], keysT[t, :, 0])
        nc.sync.dma_start(k_sb[:, 1], keysT[t, :, 1])
        nc.gpsimd.dma_start(m_sb[:], maskv[t])
        nc.gpsimd.dma_start(v_sb[:], vals[t])

        # proj_ps[:, hh, 0:Q] = q_proj; [:, hh, Q:Q+C] = k_proj
        # (per-hh region padded to PROJ_W so each stays inside one PSUM bank)
        proj_ps = ps_proj.tile([128, HC, PROJ_W], F32, tag="proj")
        for hh in range(HC):
            for dc in range(DC):
                nc.tensor.matmul(
                    proj_ps[:, hh, 0:Q],
                    lhsT=Wq_sb[:, dc, hh * 128:(hh + 1) * 128],
                    rhs=qT_sb[:, dc, :],
                    start=(dc == 0), stop=(dc == DC - 1),
                )
            for dc in range(DC):
                nc.tensor.matmul(
                    proj_ps[:, hh, Q:Q + C],
                    lhsT=Wk_sb[:, dc, hh * 128:(hh + 1) * 128],
                    rhs=k_sb[:, dc, :],
                    start=(dc == 0), stop=(dc == DC - 1),
                )
        qp_sb = proj_pool.tile([128, HC, Q], F32, tag="qp")
        kp_sb = proj_pool.tile([128, HC * C], BF16, tag="kp")
        nc.vector.tensor_copy(qp_sb[:], proj_ps[:, :, 0:Q])
        nc.vector.tensor_copy(
            kp_sb[:].rearrange("p (h c) -> p h c", h=HC),
            proj_ps[:, :, Q:Q + C])
        return k_sb, qT_sb, v_sb, m_sb, qp_sb, kp_sb

    state = {}

    for t in range(n_tasks):
        if t == 0:
            state[0] = emit_inputs_and_proj(0)
        _, _, v_sb, m_sb, qp_sb, kp_sb = state.pop(t)
        if t + 1 < n_tasks:
            # Pipelined: next task's projections go ahead of this task's
            # scores in the PE stream, so PE/DVE never stall at task turnover.
            state[t + 1] = emit_inputs_and_proj(t + 1)

        # ---- qk broadcast-add (DVE) + tanh (ACT), flat tiles ----
        if t == 0:
            group_lens = [4, 12] + [GQ] * ((Q - GQ) // GQ)
        elif t == n_tasks - 1:
            # taper the final groups so the last task's scores/exp tail is short
            group_lens = [GQ] * ((Q - GQ) // GQ) + [GQ - 8, 4, 4]
        else:
            group_lens = [GQ] * (Q // GQ)
        feats = []   # (flat feat tile, local idx) per query
        q0 = 0
        for ln in group_lens:
            qk = qk_pool.tile([128, GQ * HC * C], BF16, tag="qk")
            for i in range(ln):
                qq = q0 + i
                for hh in range(HC):
                    nc.vector.tensor_scalar_add(
                        qk[:, (i * HC + hh) * C:(i * HC + hh + 1) * C],
                        kp_sb[:, hh * C:(hh + 1) * C],
                        qp_sb[:, hh, qq:qq + 1],
                    )
            feat = feat_pool.tile([128, GQ * HC * C], BF16, tag="feat")
            nc.scalar.activation(feat[:, 0:ln * HC * C], qk[:, 0:ln * HC * C],
                                 mybir.ActivationFunctionType.Tanh)
            for i in range(ln):
                feats.append((feat, i))
            q0 += ln

        # ---- scoresT[c, q] (PE): feat as stationary, w_v streaming ----
        # sc_ps: [:, ch*Q + q] = scoresT chunk ch; [0:1, CH*Q:] = z row
        sc_ps = ps_sc.tile([128, (CH + 1) * Q], F32, tag="sc")
        for qq in range(Q):
            ftile, i = feats[qq]
            for ch in range(CH):
                for hh in range(HC):
                    off = (i * HC + hh) * C + ch * 128
                    nc.tensor.matmul(
                        sc_ps[:, ch * Q + qq:ch * Q + qq + 1],
                        lhsT=ftile[:, off:off + 128],
                        rhs=wv_sb[:, hh:hh + 1],
                        start=(hh == 0), stop=(hh == HC - 1),
                    )

        # ---- exp (ACT) ----
        p_sb = p_pool.tile([128, CH * Q], F32, tag="p")
        nc.scalar.activation(p_sb[:], sc_ps[:, 0:CH * Q],
                             mybir.ActivationFunctionType.Exp)

        # ---- o = V.T @ p, z = mask.T @ p (PE, accumulate over ch) ----
        o_ps = ps_o.tile([128, DC, Q], F32, tag="o")
        for dc in range(DC):
            for ch in range(CH):
                nc.tensor.matmul(
                    o_ps[:, dc, :],
                    lhsT=v_sb[:, ch, dc * 128:(dc + 1) * 128],
                    rhs=p_sb[:, ch * Q:(ch + 1) * Q],
                    start=(ch == 0), stop=(ch == CH - 1),
                )
        for ch in range(CH):
            nc.tensor.matmul(
                sc_ps[0:1, CH * Q:(CH + 1) * Q],
                lhsT=m_sb[:, ch:ch + 1],
                rhs=p_sb[:, ch * Q:(ch + 1) * Q],
                start=(ch == 0), stop=(ch == CH - 1),
            )

        # ---- evacuate + output DMA ----
        o_sb = out_pool.tile([128, DC, Q], F32, tag="osb")
        s_sb = out_pool.tile([1, Q], F32, tag="ssb")
        nc.vector.tensor_copy(o_sb[:], o_ps[:])
        nc.vector.tensor_copy(s_sb[:], sc_ps[0:1, CH * Q:(CH + 1) * Q])
        nc.sync.dma_start(o_out[t], o_sb[:])
        nc.sync.dma_start(s_out[t], s_sb[:])


_NC_CACHE = {}


def build_nc(n_tasks):
    if n_tasks in _NC_CACHE:
        return _NC_CACHE[n_tasks]
    nc = bacc.Bacc("TRN2", target_bir_lowering=False, debug=False)
    aps = {
        "keysT": nc.dram_tensor("keysT", [n_tasks, 128, DC, C], F32,
                                kind="ExternalInput").ap(),
        "queriesT": nc.dram_tensor("queriesT", [n_tasks, 128, DC, Q], F32,
                                   kind="ExternalInput").ap(),
        "vals": nc.dram_tensor("vals", [n_tasks, 128, CH, D], F32,
                               kind="ExternalInput").ap(),
        "maskv": nc.dram_tensor("maskv", [n_tasks, 128, CH], F32,
                                kind="ExternalInput").ap(),
        "Wq": nc.dram_tensor("Wq", [128, DC, H], F32, kind="ExternalInput").ap(),
        "Wk": nc.dram_tensor("Wk", [128, DC, H], F32, kind="ExternalInput").ap(),
        "wv": nc.dram_tensor("wv", [128, HC], BF16, kind="ExternalInput").ap(),
        "o_out": nc.dram_tensor("o_out", [n_tasks, 128, DC, Q], F32,
                                kind="ExternalOutput").ap(),
        "s_out": nc.dram_tensor("s_out", [n_tasks, 1, Q], F32,
                                kind="ExternalOutput").ap(),
    }
    with tile.TileContext(nc) as tc:
        with ExitStack() as stack:
            tc.ctx = stack
            emit_kernel(tc, aps, n_tasks)
    nc.compile()
    _NC_CACHE[n_tasks] = (nc, aps)
    return nc, aps


def make_task_list(valid_lens):
    """Uniform (b, c0) tasks; None = dummy task."""
    chunks = []
    for b in range(B):
        v = int(valid_lens[b])
        for c0 in range(0, v, C):
            chunks.append((b, c0))
    n_tasks = math.ceil(len(chunks) / N_CORES)
    chunks += [None] * (n_tasks * N_CORES - len(chunks))
    per_core = [chunks[i * n_tasks:(i + 1) * n_tasks] for i in range(N_CORES)]
    return per_core, n_tasks


def pack_inputs(queries, keys, values, valid_lens, W_q, W_k, w_v, per_core, n_tasks):
    """Build the per-core input maps (host-side layout only)."""
    Wq_arr = np.ascontiguousarray(
        W_q.reshape(DC, 128, H).transpose(1, 0, 2))       # [128, DC, H]
    Wk_arr = np.ascontiguousarray(
        W_k.reshape(DC, 128, H).transpose(1, 0, 2))
    wv_arr = np.ascontiguousarray(
        w_v.reshape(HC, 128).T.astype(ml_dtypes.bfloat16))  # [128, HC]

    in_maps = []
    for core in range(N_CORES):
        keysT = np.zeros((n_tasks, 128, DC, C), np.float32)
        queriesT = np.zeros((n_tasks, 128, DC, Q), np.float32)
        vals = np.zeros((n_tasks, 128, CH, D), np.float32)
        maskv = np.zeros((n_tasks, 128, CH), np.float32)
        for t, task in enumerate(per_core[core]):
            if task is None:
                continue
            b, c0 = task
            v = int(valid_lens[b])
            n = min(C, v - c0)
            kT = np.zeros((D, C), np.float32)
            kT[:, :n] = keys[b, c0:c0 + n, :].T
            keysT[t] = kT.reshape(DC, 128, C).transpose(1, 0, 2)
            qT = queries[b].T.reshape(DC, 128, Q)
            queriesT[t] = qT.transpose(1, 0, 2)
            vv = np.zeros((C, D), np.float32)
            vv[:n] = values[b, c0:c0 + n, :]
            vals[t] = vv.reshape(CH, 128, D).transpose(1, 0, 2)
            mm = np.zeros(C, np.float32)
            mm[:n] = 1.0
            maskv[t] = mm.reshape(CH, 128).T
        in_maps.append({
            "keysT": keysT, "queriesT": queriesT, "vals": vals, "maskv": maskv,
            "Wq": Wq_arr, "Wk": Wk_arr, "wv": wv_arr,
        })
    return in_maps


def combine_outputs(results, per_core, valid_lens):
    o_acc = np.zeros((B, D, Q), np.float64)
    s_acc = np.zeros((B, Q), np.float64)
    for core in range(N_CORES):
        o = results[core]["o_out"]   # [T, 128, DC, Q]
        s = results[core]["s_out"]   # [T, 1, Q]
        for t, task in enumerate(per_core[core]):
            if task is None:
                continue
            b, _ = task
            o_acc[b] += o[t].transpose(1, 0, 2).reshape(D, Q)
            s_acc[b] += s[t][0]
    out = o_acc / s_acc[:, None, :]          # [B, D, Q]
    return np.ascontiguousarray(out.transpose(0, 2, 1)).astype(np.float32)


def kernel(queries, keys, values, valid_lens, W_q, W_k, w_v, _run_kwargs=None):
    queries = np.asarray(queries, np.float32)
    keys = np.asarray(keys, np.float32)
    values = np.asarray(values, np.float32)
    valid_lens = np.asarray(valid_lens)
    W_q = np.asarray(W_q, np.float32)
    W_k = np.asarray(W_k, np.float32)
    w_v = np.asarray(w_v, np.float32)

    per_core, n_tasks = make_task_list(valid_lens)
    nc, _ = build_nc(n_tasks)
    in_maps = pack_inputs(queries, keys, values, valid_lens, W_q, W_k, w_v,
                          per_core, n_tasks)
    kw = dict(_run_kwargs or {})
    res = bass_utils.run_bass_kernel_spmd(nc, in_maps, list(range(N_CORES)), **kw)
    out = combine_outputs(res.results, per_core, valid_lens)
    if _run_kwargs is not None:
        kernel._last_result = res
    return out
